# revision 22
# baseline (speedup 1.0000x reference)
"""AFMADE block kernel for 8 Trainium2 NeuronCores.

Strategy (data-parallel over batch, per sharding hint):
 - Shard x [2048, 16] into 8 batch shards of 256; replicate weights.
 - Host precomputes masked weight-normalized weights (float64), permutes
   hidden units by MADE degree (makes the hidden-hidden mask block
   triangular so ~1/3 of the H x H matmul tiles are exactly zero and are
   skipped), folds biases and the elu "+1" shift into weights/biases.
 - Device runs the 16-step recurrence in feature-major layout
   ([features, batch] on-chip) so layers chain without transposes.
   Matmuls run in float32r (4x the fp32 rate at N>=256).
 - ELU uses the exact identity elu(z)+1 = max(z+1, min(exp(z), 1)):
   one ScalarE exp + one fused VectorE scalar_tensor_tensor per layer.
 - Epilogue: y_new = (x - mu) * exp(-logstd) (the reference's +1e-12 in
   the denominator is ~1e-12 relative and ignored).
"""

import os
import sys

_D, _H, _B = 16, 1024, 2048
_NCORES = 8
_BL = _B // _NCORES  # batch per core

_KT = _H // 128  # 8 k-tiles
_MT = _H // 128  # 8 m-tiles


def _ensure_paths():
    for p in ("/opt/pypackages", "/opt/trn_rl_repo"):
        if p not in sys.path:
            sys.path.insert(0, p)
    # bass2jax needs the axon jax platform; a cpu-only pin would break it
    if os.environ.get("JAX_PLATFORMS") in ("cpu",):
        os.environ.pop("JAX_PLATFORMS")


# ---------------------------------------------------------------- host math


def _masks_and_perm():
    import numpy as np

    deg_in = np.arange(1, _D + 1)
    deg_h = np.arange(_H) % (_D - 1) + 1
    m0 = (deg_h[:, None] >= deg_in[None, :]).astype(np.float64)
    m1 = (deg_h[:, None] >= deg_h[None, :]).astype(np.float64)
    m2 = (deg_in[:, None] > deg_h[None, :]).astype(np.float64)
    hperm = np.argsort(deg_h, kind="stable")
    deg_sorted = deg_h[hperm]
    return m0, m1, m2, hperm, deg_sorted


def _keep_table(deg_sorted):
    """keep[m][k]: does W1p[128m:128m+128, 128k:128k+128] have any nonzero?
    (mask is deg_out >= deg_in after degree sorting)"""
    import numpy as np

    dmin = [int(deg_sorted[k * 128 : k * 128 + 128].min()) for k in range(_KT)]
    dmax = [int(deg_sorted[m * 128 : m * 128 + 128].max()) for m in range(_MT)]
    return [[dmax[m] >= dmin[k] for k in range(_KT)] for m in range(_MT)]


def _wn(v, g, m):
    import numpy as np

    v = np.asarray(v, np.float64)
    g = np.asarray(g, np.float64)
    return m * (g[:, None] * v / np.linalg.norm(v, axis=1, keepdims=True))


def _preprocess(inputs):
    """All weight math in float64 on host; returns per-network packed
    fp32 tensors in the on-chip layouts."""
    import numpy as np

    m0, m1, m2, hperm, deg_sorted = _masks_and_perm()
    nets = {}
    for n in ("mu", "lv"):
        W0 = _wn(inputs[f"{n}_v0"], inputs[f"{n}_g0"], m0)[hperm, :]  # [H, D]
        W1 = _wn(inputs[f"{n}_v1"], inputs[f"{n}_g1"], m1)[hperm][:, hperm]
        W2 = _wn(inputs[f"{n}_v2"], inputs[f"{n}_g2"], m2)[:, hperm]  # [D, H]
        b0 = np.asarray(inputs[f"{n}_b0"], np.float64)[hperm]
        b1 = np.asarray(inputs[f"{n}_b1"], np.float64)[hperm]
        b2 = np.asarray(inputs[f"{n}_b2"], np.float64)
        # downstream consumes h' = elu(z) + 1, so subtract W @ 1
        b1_eff = b1 - W1.sum(axis=1)
        b2_eff = b2 - W2.sum(axis=1)
        # matmuls produce z' = z + 1 directly
        b0p = b0 + 1.0
        b1p = b1_eff + 1.0

        # w0t_aug [17, 1024]: rows 0..15 = W0.T, row 16 = b0p (ones-row trick)
        w0t = np.empty((_D + 1, _H), np.float64)
        w0t[:_D] = W0.T
        w0t[_D] = b0p
        # w1t packed [128, KT*MT*128]: block (k, m) = W1[128m:.., 128k:..].T
        w1t = np.ascontiguousarray(
            W1.reshape(_MT, 128, _KT, 128).transpose(3, 2, 0, 1)
        ).reshape(128, _KT * _MT * 128)
        # w1t columns: k*1024 + m*128 + j
        # b1 row [1, 1024]
        b1r = b1p.reshape(1, _H)
        # w2t [128, KT*16]: block k = W2[:, 128k:..].T
        w2t = np.ascontiguousarray(W2.reshape(_D, _KT, 128).transpose(2, 1, 0)).reshape(
            128, _KT * _D
        )
        nets[n] = dict(
            w0t=w0t.astype(np.float32),
            w1t=w1t.astype(np.float32),
            b1r=b1r.astype(np.float32),
            w2t=w2t.astype(np.float32),
            b2_eff=b2_eff,
        )
    return nets


# ---------------------------------------------------------------- builder


def _build(nc):
    from contextlib import ExitStack

    import concourse.mybir as mybir
    import concourse.tile as tile

    f32 = mybir.dt.float32
    f32r = mybir.dt.float32r
    AF = mybir.ActivationFunctionType
    OP = mybir.AluOpType

    _, _, _, _, deg_sorted = _masks_and_perm()
    keep = _keep_table(deg_sorted)

    NETS = ("mu", "lv")
    d_xb = nc.dram_tensor("xb", [_D, _BL], f32, kind="ExternalInput")
    d_y0 = nc.dram_tensor("y0", [_D + 1, _BL], f32r, kind="ExternalInput")
    d_ones = nc.dram_tensor("ones_c", [_D, _BL], f32r, kind="ExternalInput")
    d_ones32 = nc.dram_tensor("ones32", [_D, 1], f32, kind="ExternalInput")
    d_eb = nc.dram_tensor("ebias", [_D, 1], f32, kind="ExternalInput")
    d_hb = nc.dram_tensor("hbias", [_D, 1], f32, kind="ExternalInput")
    d_w0 = {n: nc.dram_tensor(f"w0t_{n}", [_D + 1, _H], f32r, kind="ExternalInput") for n in NETS}
    d_w1 = {n: nc.dram_tensor(f"w1t_{n}", [128, _KT * _MT * 128], f32r, kind="ExternalInput") for n in NETS}
    d_b1 = {n: nc.dram_tensor(f"b1r_{n}", [1, _H], f32r, kind="ExternalInput") for n in NETS}
    d_w2 = {n: nc.dram_tensor(f"w2t_{n}", [128, _KT * _D], f32r, kind="ExternalInput") for n in NETS}
    d_oy = nc.dram_tensor("out_y", [_D, _BL], f32r, kind="ExternalOutput")
    d_ols = nc.dram_tensor("out_ls", [1, _BL], f32, kind="ExternalOutput")

    with tile.TileContext(nc) as tc, ExitStack() as ctx:
        singles = ctx.enter_context(tc.tile_pool(name="singles", bufs=1))
        state = ctx.enter_context(tc.tile_pool(name="state", bufs=1))
        work = ctx.enter_context(tc.tile_pool(name="work", bufs=3))
        epool = ctx.enter_context(tc.tile_pool(name="epool", bufs=2))
        ppool = ctx.enter_context(tc.tile_pool(name="ppool", bufs=2, space="PSUM"))

        # ---- weight loads (k-major order so the first L1 tiles land early)
        w0 = {}
        w1 = {}
        b1 = {}
        w2 = {}
        xb = singles.tile([_D, _BL], f32, name="xb_s")
        nc.sync.dma_start(out=xb, in_=d_xb[:, :])
        eb = singles.tile([_D, 1], f32, name="eb_s")
        nc.sync.dma_start(out=eb, in_=d_eb[:, :])
        hb = singles.tile([_D, 1], f32, name="hb_s")
        nc.sync.dma_start(out=hb, in_=d_hb[:, :])
        for n in NETS:
            w0[n] = singles.tile([_D + 1, _H], f32r, name=f"w0_{n}")
            nc.sync.dma_start(out=w0[n], in_=d_w0[n][:, :])
            b1[n] = singles.tile([1, _H], f32r, name=f"b1_{n}")
            nc.sync.dma_start(out=b1[n], in_=d_b1[n][:, :])
            w2[n] = singles.tile([128, _KT * _D], f32r, name=f"w2_{n}")
            nc.sync.dma_start(out=w2[n], in_=d_w2[n][:, :])
            w1[n] = singles.tile([128, _KT * _MT * 128], f32r, name=f"w1_{n}")
        for k in range(_KT):  # interleave mu/lv per k so both nets start early
            for n in NETS:
                nc.sync.dma_start(
                    out=w1[n][:, k * 1024 : (k + 1) * 1024],
                    in_=d_w1[n][:, k * 1024 : (k + 1) * 1024],
                )

        # ---- persistent activations
        yT = state.tile([_D + 1, _BL], f32r, name="yT")  # row 16 = ones
        h0 = {n: state.tile([128, _KT * _BL], f32r, name=f"h0_{n}") for n in NETS}
        h1 = {n: state.tile([128, _KT * _BL], f32r, name=f"h1_{n}") for n in NETS}
        ones_t = state.tile([_D, _BL], f32r, name="ones_t")
        ones32 = state.tile([_D, 1], f32, name="ones32_s")
        nc.sync.dma_start(out=ones32, in_=d_ones32[:, :])
        negone = state.tile([128, 1], f32, name="negone")
        # f32r tiles can't be memset; DMA the constants. yT's row 16 must
        # come via DMA anyway (compute engines can't base at partition 16).
        nc.sync.dma_start(out=yT[:, :], in_=d_y0[:, :])
        nc.sync.dma_start(out=ones_t[:, :], in_=d_ones[:, :])
        ones16 = ones_t[:, 0:1]
        onesrow = ones_t[0:1, :]
        nc.vector.memset(negone, -1.0)

        NCHUNK = 2  # elementwise chunking of [128, 2048] ops
        CW = _MT * _BL // NCHUNK

        def elu(z, h, n, lay):
            """h = max(z, min(exp(z-1), 1)) elementwise, [128, MT*BL]"""
            for c in range(NCHUNK):
                sl = slice(c * CW, (c + 1) * CW)
                E = epool.tile([128, CW], f32, name=f"E_{n}{lay}{c}", tag="E")
                nc.scalar.activation(E, z[:, sl], AF.Exp, bias=negone[:, 0:1], scale=1.0)
                nc.vector.scalar_tensor_tensor(
                    out=h[:, sl], in0=E, scalar=1.0, in1=z[:, sl],
                    op0=OP.min, op1=OP.max,
                )

        for step in range(_D):
            # psum slot rotation: mu chain on slot A, lv chain on slot B
            z0 = {}
            z1 = {}
            z2 = {}
            for n in NETS:
                z0[n] = ppool.tile([128, _MT * _BL], f32, name=f"z0_{n}_{step}", tag="zbig")
            for n in NETS:
                z1[n] = ppool.tile([128, _MT * _BL], f32, name=f"z1_{n}_{step}", tag="zbig")
            for n in NETS:
                z2[n] = ppool.tile([_D, _BL], f32, name=f"z2_{n}_{step}", tag="zbig")

            # L0: z0 = W0aug.T @ [y; 1]  (K = 17)
            for n in NETS:
                for m in range(_MT):
                    nc.tensor.matmul(
                        z0[n][:, m * _BL : (m + 1) * _BL],
                        lhsT=w0[n][:, m * 128 : (m + 1) * 128].bitcast(f32r),
                        rhs=yT[:, :].bitcast(f32r),
                        start=True,
                        stop=True,
                    )
            for n in NETS:
                elu(z0[n], h0[n], n, 0)

            # L1: z1 = W1.T @ h0 + b1 (bias via ones-row matmul, K=1)
            for n in NETS:
                for m in range(_MT):
                    nc.tensor.matmul(
                        z1[n][:, m * _BL : (m + 1) * _BL],
                        lhsT=b1[n][0:1, m * 128 : (m + 1) * 128].bitcast(f32r),
                        rhs=onesrow[:, :].bitcast(f32r),
                        start=True,
                        stop=False,
                    )
                    kept = [k for k in range(_KT) if keep[m][k]]
                    for i, k in enumerate(kept):
                        nc.tensor.matmul(
                            z1[n][:, m * _BL : (m + 1) * _BL],
                            lhsT=w1[n][:, k * 1024 + m * 128 : k * 1024 + (m + 1) * 128].bitcast(f32r),
                            rhs=h0[n][:, k * _BL : (k + 1) * _BL].bitcast(f32r),
                            start=False,
                            stop=(i == len(kept) - 1),
                        )
            for n in NETS:
                elu(z1[n], h1[n], n, 1)

            # L2: z2 = W2.T @ h1  (M = 16; biases folded into epilogue)
            for n in NETS:
                for k in range(_KT):
                    nc.tensor.matmul(
                        z2[n][:, :],
                        lhsT=w2[n][:, k * _D : (k + 1) * _D].bitcast(f32r),
                        rhs=h1[n][:, k * _BL : (k + 1) * _BL].bitcast(f32r),
                        start=(k == 0),
                        stop=(k == _KT - 1),
                    )

            # epilogue: y = (xb - z2_mu) * exp(-0.5 z2_lv + ebias)
            t = work.tile([_D, _BL], f32, name=f"t_{step}", tag="t")
            nc.scalar.activation(t, z2["lv"][:, :], AF.Exp, bias=eb[:, 0:1], scale=-0.5)
            dm = work.tile([_D, _BL], f32, name=f"d_{step}", tag="d")
            nc.vector.tensor_sub(dm, xb, z2["mu"][:, :])
            if step == _D - 1:
                # full-fp32 sum (4 cyc/row, one-time): f32r rounding of the
                # small logstd values would cost ~1e-2 relative after the
                # cancelling sum over D
                ls = work.tile([_D, _BL], f32, name="ls", tag="ls")
                nc.vector.tensor_scalar(
                    out=ls, in0=z2["lv"][:, :], scalar1=0.5, scalar2=hb[:, 0:1],
                    op0=OP.mult, op1=OP.add,
                )
                zls = ppool.tile([1, _BL], f32, name="zls", tag="zbig")
                nc.tensor.matmul(
                    zls, lhsT=ones32[:, :], rhs=ls[:, :],
                    start=True, stop=True,
                )
                ls_sb = work.tile([1, _BL], f32, name="ls_sb", tag="t")
                nc.vector.tensor_copy(ls_sb, zls)
                nc.sync.dma_start(out=d_ols[:, :], in_=ls_sb)
            nc.vector.tensor_mul(yT[: _D, :], dm, t)

        nc.sync.dma_start(out=d_oy[:, :], in_=yT[: _D, :])
    return nc


# ---------------------------------------------------------------- entry


def kernel(**inputs):
    _ensure_paths()
    import numpy as np

    from concourse import bacc
    from concourse.bass_utils import run_bass_kernel_spmd

    nets = _preprocess(inputs)
    x = np.asarray(inputs["x"], np.float32)  # [B, D]

    b2mu = nets["mu"].pop("b2_eff")
    b2lv = nets["lv"].pop("b2_eff")
    ebias = (-0.5 * b2lv).astype(np.float32).reshape(_D, 1)
    hbias = (0.5 * b2lv).astype(np.float32).reshape(_D, 1)

    nc = bacc.Bacc()
    _build(nc)
    nc.finalize()

    y0 = np.zeros((_D + 1, _BL), np.float32)
    y0[_D, :] = 1.0
    ones_c = np.ones((_D, _BL), np.float32)
    in_maps = []
    for c in range(_NCORES):
        shard = x[c * _BL : (c + 1) * _BL, :]  # [BL, D]
        xb = np.ascontiguousarray(shard.T) - b2mu.astype(np.float32)[:, None]
        m = {
            "xb": np.ascontiguousarray(xb, np.float32),
            "y0": y0,
            "ones_c": ones_c,
            "ones32": np.ones((_D, 1), np.float32),
            "ebias": ebias,
            "hbias": hbias,
        }
        for n in ("mu", "lv"):
            m[f"w0t_{n}"] = nets[n]["w0t"]
            m[f"w1t_{n}"] = nets[n]["w1t"]
            m[f"b1r_{n}"] = nets[n]["b1r"]
            m[f"w2t_{n}"] = nets[n]["w2t"]
        in_maps.append(m)

    res = run_bass_kernel_spmd(nc, in_maps, core_ids=list(range(_NCORES)))
    ys = []
    lss = []
    for c in range(_NCORES):
        ys.append(np.asarray(res.results[c]["out_y"]).T)  # [BL, D]
        lss.append(np.asarray(res.results[c]["out_ls"])[0])  # [BL]
    y = np.concatenate(ys, axis=0).astype(np.float32)
    ls = np.concatenate(lss, axis=0).astype(np.float32)
    return y, ls


# revision 26
# speedup vs baseline: 1.6371x; 1.6371x over previous
"""AFMADE block kernel for 8 Trainium2 NeuronCores.

Strategy (data-parallel over batch, per the sharding hint):
 - Shard x [2048, 16] into 8 batch shards of 256; replicate the weights.
 - Host precomputes masked weight-normalized weights in float64, permutes
   hidden units by MADE degree (makes the hidden-hidden mask block
   triangular, so 21/64 of the H x H matmul tiles are exactly zero and
   are skipped), and folds biases plus the elu "+1" shift into
   weights/biases.
 - Device runs the 16-step recurrence in feature-major layout
   ([features, batch] on-chip) so the three layers chain without any
   transposes.
 - Mixed precision: the mu network runs in bf16 (halves its weight-load
   time, which is the kernel's bottleneck); the lv network runs in
   float32r (~4x faster than fp32 matmul at N>=256) because the summed
   logstd output cancels heavily and needs the extra bits.
 - ELU via the exact identity elu(z)+1 = max(z+1, min(exp(z), 1)):
   one ScalarE exp + one fused VectorE scalar_tensor_tensor per chunk.
 - L1 biases ride rank-2 indicator matmuls (K=2, N=512): two bias rows
   against a constant [2, 512] 0/1 indicator, one matmul per PSUM bank.
 - Epilogue: y_new = (x - mu) * exp(-logstd); the reference's +1e-12 in
   the denominator is a ~1e-12 relative perturbation and is dropped.
"""

import os
import sys

_D, _H, _B = 16, 1024, 2048
_NCORES = 8
_BL = _B // _NCORES  # batch per core

_KT = _H // 128  # 8 k-tiles
_MT = _H // 128  # 8 m-tiles
_NCH = 4  # psum/elu chunks per [128, 2048] layer tensor
_CW = _MT * _BL // _NCH  # chunk width (512)


def _ensure_paths():
    for p in ("/opt/pypackages", "/opt/trn_rl_repo"):
        if p not in sys.path:
            sys.path.insert(0, p)
    # bass2jax needs the axon jax platform; a cpu-only pin would break it
    if os.environ.get("JAX_PLATFORMS") in ("cpu",):
        os.environ.pop("JAX_PLATFORMS")


# ---------------------------------------------------------------- host math


def _masks_and_perm():
    import numpy as np

    deg_in = np.arange(1, _D + 1)
    deg_h = np.arange(_H) % (_D - 1) + 1
    m0 = (deg_h[:, None] >= deg_in[None, :]).astype(np.float64)
    m1 = (deg_h[:, None] >= deg_h[None, :]).astype(np.float64)
    m2 = (deg_in[:, None] > deg_h[None, :]).astype(np.float64)
    hperm = np.argsort(deg_h, kind="stable")
    deg_sorted = deg_h[hperm]
    return m0, m1, m2, hperm, deg_sorted


def _keep_table(deg_sorted):
    """keep[m][k]: does W1p[128m:128m+128, 128k:128k+128] have any
    nonzero? (mask is deg_out >= deg_in after degree sorting)"""
    dmin = [int(deg_sorted[k * 128 : k * 128 + 128].min()) for k in range(_KT)]
    dmax = [int(deg_sorted[m * 128 : m * 128 + 128].max()) for m in range(_MT)]
    return [[dmax[m] >= dmin[k] for k in range(_KT)] for m in range(_MT)]


def _wn(v, g, m):
    import numpy as np

    v = np.asarray(v, np.float64)
    g = np.asarray(g, np.float64)
    return m * (g[:, None] * v / np.linalg.norm(v, axis=1, keepdims=True))


def _preprocess(inputs):
    """All weight math in float64 on host; returns per-network packed
    tensors in the on-chip layouts (numpy f32; dtype cast happens at
    in_map build)."""
    import numpy as np

    m0, m1, m2, hperm, _ = _masks_and_perm()
    nets = {}
    for n in ("mu", "lv"):
        W0 = _wn(inputs[f"{n}_v0"], inputs[f"{n}_g0"], m0)[hperm, :]  # [H, D]
        W1 = _wn(inputs[f"{n}_v1"], inputs[f"{n}_g1"], m1)[hperm][:, hperm]
        W2 = _wn(inputs[f"{n}_v2"], inputs[f"{n}_g2"], m2)[:, hperm]  # [D, H]
        b0 = np.asarray(inputs[f"{n}_b0"], np.float64)[hperm]
        b1 = np.asarray(inputs[f"{n}_b1"], np.float64)[hperm]
        b2 = np.asarray(inputs[f"{n}_b2"], np.float64)
        # downstream consumes h' = elu(z) + 1, so pre-subtract W @ 1
        b1_eff = b1 - W1.sum(axis=1)
        b2_eff = b2 - W2.sum(axis=1)
        # matmuls produce z' = z + 1 directly
        b0p = b0 + 1.0
        b1p = b1_eff + 1.0

        # w0t_aug [17, 1024]: rows 0..15 = W0.T, row 16 = b0p (ones-row)
        w0t = np.empty((_D + 1, _H), np.float64)
        w0t[:_D] = W0.T
        w0t[_D] = b0p
        # w1t packed [128, KT*MT*128]: block (k, m) = W1[128m:, 128k:].T
        w1t = np.ascontiguousarray(
            W1.reshape(_MT, 128, _KT, 128).transpose(3, 2, 0, 1)
        ).reshape(128, _KT * _MT * 128)
        # b1 packed for the K=2 indicator matmul: b1pk[i, 128b+p] =
        # b1p[128*(2b+i)+p]  -> lhsT slice [2, 128] per psum bank b
        b1pk = np.ascontiguousarray(
            b1p.reshape(4, 2, 128).transpose(1, 0, 2)
        ).reshape(2, 512)
        # w2t [128, KT*16]: block k = W2[:, 128k:].T
        w2t = np.ascontiguousarray(
            W2.reshape(_D, _KT, 128).transpose(2, 1, 0)
        ).reshape(128, _KT * _D)
        nets[n] = dict(w0t=w0t, w1t=w1t, b1pk=b1pk, w2t=w2t, b2_eff=b2_eff)
    return nets


# ---------------------------------------------------------------- builder


def _build(nc):
    from contextlib import ExitStack

    import concourse.mybir as mybir
    import concourse.tile as tile

    f32 = mybir.dt.float32
    f32r = mybir.dt.float32r
    bf16 = mybir.dt.bfloat16
    AF = mybir.ActivationFunctionType
    OP = mybir.AluOpType

    _, _, _, _, deg_sorted = _masks_and_perm()
    keep = _keep_table(deg_sorted)

    NETS = ("mu", "lv")
    DT = {"mu": bf16, "lv": f32r}

    d_xb = nc.dram_tensor("xb", [_D, _BL], f32, kind="ExternalInput")
    d_y0r = nc.dram_tensor("y0r", [_D + 1, _BL], f32r, kind="ExternalInput")
    d_y0b = nc.dram_tensor("y0b", [_D + 1, _BL], bf16, kind="ExternalInput")
    # ind2: rows = bank-half indicators ([1..]=0/1 pattern); col j of row i
    # is 1.0 if (j // 256) == i else 0.0
    d_indr = nc.dram_tensor("indr", [2, _CW], f32r, kind="ExternalInput")
    d_indb = nc.dram_tensor("indb", [2, _CW], bf16, kind="ExternalInput")
    d_ones32 = nc.dram_tensor("ones32", [_D, 1], f32, kind="ExternalInput")
    d_eb = nc.dram_tensor("ebias", [_D, 1], f32, kind="ExternalInput")
    d_hb = nc.dram_tensor("hbias", [_D, 1], f32, kind="ExternalInput")
    d_w0 = {n: nc.dram_tensor(f"w0t_{n}", [_D + 1, _H], DT[n], kind="ExternalInput") for n in NETS}
    d_w1 = {n: nc.dram_tensor(f"w1t_{n}", [128, _KT * _MT * 128], DT[n], kind="ExternalInput") for n in NETS}
    d_b1 = {n: nc.dram_tensor(f"b1pk_{n}", [2, _CW], DT[n], kind="ExternalInput") for n in NETS}
    d_w2 = {n: nc.dram_tensor(f"w2t_{n}", [128, _KT * _D], DT[n], kind="ExternalInput") for n in NETS}
    d_oy = nc.dram_tensor("out_y", [_D, _BL], f32r, kind="ExternalOutput")
    d_ols = nc.dram_tensor("out_ls", [1, _BL], f32, kind="ExternalOutput")

    with tile.TileContext(nc) as tc, ExitStack() as ctx:
        singles = ctx.enter_context(tc.tile_pool(name="singles", bufs=1))
        state = ctx.enter_context(tc.tile_pool(name="state", bufs=1))
        work = ctx.enter_context(tc.tile_pool(name="work", bufs=3))
        epool = ctx.enter_context(tc.tile_pool(name="epool", bufs=4))
        ppool = ctx.enter_context(tc.tile_pool(name="ppool", bufs=8, space="PSUM"))

        # ---- small constants first (also feed the PE warm-up)
        xb = singles.tile([_D, _BL], f32, name="xb_s")
        nc.sync.dma_start(out=xb, in_=d_xb[:, :])
        eb = singles.tile([_D, 1], f32, name="eb_s")
        nc.sync.dma_start(out=eb, in_=d_eb[:, :])
        hb = singles.tile([_D, 1], f32, name="hb_s")
        nc.sync.dma_start(out=hb, in_=d_hb[:, :])
        ones32 = singles.tile([_D, 1], f32, name="ones32_s")
        nc.sync.dma_start(out=ones32, in_=d_ones32[:, :])
        ind = {}
        b1 = {}
        w0 = {}
        w2 = {}
        w1 = {}
        for n, dd in (("lv", d_indr), ("mu", d_indb)):
            ind[n] = singles.tile([2, _CW], DT[n], name=f"ind_{n}")
            nc.sync.dma_start(out=ind[n], in_=dd[:, :])
        for n in NETS:
            b1[n] = singles.tile([2, _CW], DT[n], name=f"b1_{n}")
            nc.sync.dma_start(out=b1[n], in_=d_b1[n][:, :])
            w0[n] = singles.tile([_D + 1, _H], DT[n], name=f"w0_{n}")
            nc.sync.dma_start(out=w0[n], in_=d_w0[n][:, :])
            w2[n] = singles.tile([128, _KT * _D], DT[n], name=f"w2_{n}")
            nc.sync.dma_start(out=w2[n], in_=d_w2[n][:, :])
            w1[n] = singles.tile([128, _KT * _MT * 128], DT[n], name=f"w1_{n}")
        for k in range(_KT):  # k-interleaved so both nets can start early
            for n in NETS:
                nc.sync.dma_start(
                    out=w1[n][:, k * 1024 : (k + 1) * 1024],
                    in_=d_w1[n][:, k * 1024 : (k + 1) * 1024],
                )

        # ---- persistent activations (two yT copies: f32r for lv, bf16
        # for mu; the gpsimd engine keeps the bf16 one in sync)
        yT = {"lv": state.tile([_D + 1, _BL], f32r, name="yT_r"),
              "mu": state.tile([_D + 1, _BL], bf16, name="yT_b")}
        nc.sync.dma_start(out=yT["lv"][:, :], in_=d_y0r[:, :])
        nc.sync.dma_start(out=yT["mu"][:, :], in_=d_y0b[:, :])
        h0 = {n: state.tile([128, _KT * _BL], DT[n], name=f"h0_{n}") for n in NETS}
        h1 = {n: state.tile([128, _KT * _BL], DT[n], name=f"h1_{n}") for n in NETS}
        negone = state.tile([128, 1], f32, name="negone")
        nc.vector.memset(negone, -1.0)

        # ---- PE warm-up: ~40 junk matmuls on the small constant tiles
        # while the big w1 DMAs land, so HAM unthrottles before step 0.
        # Result feeds out_ls additively as exact zero (b1pk*ind2 sums
        # times 0.0 scalar) -- keeps DCE away without changing output.
        warm = ppool.tile([128, _CW], f32, name="warm", tag="pz")
        for i in range(40):
            nc.tensor.matmul(
                warm[:, :],
                lhsT=b1["lv"][:, 0:128],
                rhs=ind["lv"][:, :],
                start=(i == 0),
                stop=(i == 39),
            )
        warm_s = singles.tile([1, _BL], f32, name="warm_s")
        nc.vector.tensor_scalar(
            out=warm_s, in0=warm[0:1, : _BL], scalar1=0.0, scalar2=None,
            op0=OP.mult,
        )

        def elu(n, zchunks, h, lay):
            """h[:, chunk] = max(z, min(exp(z-1), 1)) per chunk."""
            out = []
            for c, z in enumerate(zchunks):
                sl = slice(c * _CW, (c + 1) * _CW)
                E = epool.tile([128, _CW], f32, name=f"E_{n}{lay}{c}", tag="E")
                a = nc.scalar.activation(E, z[:, :], AF.Exp, bias=negone[:, 0:1], scale=1.0)
                s = nc.vector.scalar_tensor_tensor(
                    out=h[:, sl], in0=E, scalar=1.0, in1=z[:, :],
                    op0=OP.min, op1=OP.max,
                )
                out.append((a, s))
            return out

        for step in range(_D):
            z0 = {}
            z1 = {}
            z2 = {}

            # ---------- L0 (+ elu0), mu first so its L1 can start early
            for n in NETS:
                z0[n] = [
                    ppool.tile([128, _CW], f32, name=f"z0_{n}_{step}_{c}", tag="pz")
                    for c in range(_NCH)
                ]
                for m in range(_MT):  # m-tile m -> chunk m//2
                    nc.tensor.matmul(
                        z0[n][m // 2][:, (m % 2) * _BL : (m % 2 + 1) * _BL],
                        lhsT=w0[n][:, m * 128 : (m + 1) * 128],
                        rhs=yT[n][:, :],
                        start=True,
                        stop=True,
                    )
                elu(n, z0[n], h0[n], 0)

            # ---------- L1 (+ elu1): bias via K=2 indicator matmul, then
            # masked k-tiles, m-major so chunks complete in order
            for n in NETS:
                z1[n] = [
                    ppool.tile([128, _CW], f32, name=f"z1_{n}_{step}_{c}", tag="pz")
                    for c in range(_NCH)
                ]
                for c in range(_NCH):  # chunk c covers m-tiles 2c, 2c+1
                    nc.tensor.matmul(
                        z1[n][c][:, :],
                        lhsT=b1[n][:, c * 128 : (c + 1) * 128],
                        rhs=ind[n][:, :],
                        start=True,
                        stop=False,
                    )
                    for half in range(2):
                        m = 2 * c + half
                        kept = [k for k in range(_KT) if keep[m][k]]
                        for i, k in enumerate(kept):
                            nc.tensor.matmul(
                                z1[n][c][:, half * _BL : (half + 1) * _BL],
                                lhsT=w1[n][:, k * 1024 + m * 128 : k * 1024 + (m + 1) * 128],
                                rhs=h0[n][:, k * _BL : (k + 1) * _BL],
                                start=False,
                                stop=(i == len(kept) - 1),
                                skip_group_check=True,
                            )
                elu(n, z1[n], h1[n], 1)

            # ---------- L2 (M = 16; output biases folded into epilogue)
            for n in NETS:
                z2[n] = ppool.tile([_D, _BL], f32, name=f"z2_{n}_{step}", tag="pz")
                for k in range(_KT):
                    nc.tensor.matmul(
                        z2[n][:, :],
                        lhsT=w2[n][:, k * _D : (k + 1) * _D],
                        rhs=h1[n][:, k * _BL : (k + 1) * _BL],
                        start=(k == 0),
                        stop=(k == _KT - 1),
                    )

            # ---------- epilogue: y = (xb - z2_mu) * exp(-0.5 z2_lv + eb)
            t = work.tile([_D, _BL], f32, name=f"t_{step}", tag="t")
            nc.scalar.activation(t, z2["lv"][:, :], AF.Exp, bias=eb[:, 0:1], scale=-0.5)
            dm = work.tile([_D, _BL], f32, name=f"d_{step}", tag="d")
            nc.vector.tensor_sub(dm, xb, z2["mu"][:, :])
            if step == _D - 1:
                # full-fp32 logstd sum: f32r rounding of the small logstd
                # values would cost ~1e-2 relative after the cancelling sum
                ls = work.tile([_D, _BL], f32, name="ls", tag="ls")
                nc.vector.tensor_scalar(
                    out=ls, in0=z2["lv"][:, :], scalar1=0.5, scalar2=hb[:, 0:1],
                    op0=OP.mult, op1=OP.add,
                )
                zls = ppool.tile([1, _BL], f32, name="zls", tag="pz")
                nc.tensor.matmul(zls, lhsT=ones32[:, :], rhs=ls[:, :],
                                 start=True, stop=True)
                ls_sb = work.tile([1, _BL], f32, name="ls_sb", tag="t")
                # + warm_s (exact zero) keeps the warm-up matmuls live
                nc.vector.tensor_add(ls_sb, zls, warm_s)
                nc.sync.dma_start(out=d_ols[:, :], in_=ls_sb)
            nc.vector.tensor_mul(yT["lv"][: _D, :], dm, t)
            nc.gpsimd.tensor_copy(
                out=yT["mu"][: _D, :], in_=yT["lv"][: _D, :].bitcast(f32)
            )

        nc.sync.dma_start(out=d_oy[:, :], in_=yT["lv"][: _D, :])
    return nc


# ---------------------------------------------------------------- entry


def _make_in_maps(inputs):
    import numpy as np
    import ml_dtypes

    nets = _preprocess(inputs)
    x = np.asarray(inputs["x"], np.float32)
    b2mu = nets["mu"].pop("b2_eff")
    b2lv = nets["lv"].pop("b2_eff")
    ebias = (-0.5 * b2lv).astype(np.float32).reshape(_D, 1)
    hbias = (0.5 * b2lv).astype(np.float32).reshape(_D, 1)

    DTN = {"mu": ml_dtypes.bfloat16, "lv": np.float32}
    y0 = np.zeros((_D + 1, _BL), np.float64)
    y0[_D, :] = 1.0
    ind2 = np.zeros((2, _CW), np.float64)
    ind2[0, :_BL] = 1.0
    ind2[1, _BL:] = 1.0

    shared = {
        "y0r": y0.astype(np.float32),
        "y0b": y0.astype(ml_dtypes.bfloat16),
        "indr": ind2.astype(np.float32),
        "indb": ind2.astype(ml_dtypes.bfloat16),
        "ones32": np.ones((_D, 1), np.float32),
        "ebias": ebias,
        "hbias": hbias,
    }
    for n in ("mu", "lv"):
        dt = DTN[n]
        shared[f"w0t_{n}"] = nets[n]["w0t"].astype(dt)
        shared[f"w1t_{n}"] = nets[n]["w1t"].astype(dt)
        shared[f"b1pk_{n}"] = nets[n]["b1pk"].astype(dt)
        shared[f"w2t_{n}"] = nets[n]["w2t"].astype(dt)

    in_maps = []
    for c in range(_NCORES):
        shard = x[c * _BL : (c + 1) * _BL, :]  # [BL, D]
        xb = np.ascontiguousarray(shard.T).astype(np.float64) - b2mu[:, None]
        m = dict(shared)
        m["xb"] = np.ascontiguousarray(xb.astype(np.float32))
        in_maps.append(m)
    return in_maps


def kernel(**inputs):
    _ensure_paths()
    import numpy as np

    from concourse import bacc
    from concourse.bass_utils import run_bass_kernel_spmd

    nc = bacc.Bacc()
    _build(nc)
    nc.finalize()

    in_maps = _make_in_maps(inputs)
    res = run_bass_kernel_spmd(nc, in_maps, core_ids=list(range(_NCORES)))
    ys = []
    lss = []
    for c in range(_NCORES):
        ys.append(np.asarray(res.results[c]["out_y"]).T)  # [BL, D]
        lss.append(np.asarray(res.results[c]["out_ls"])[0])  # [BL]
    y = np.concatenate(ys, axis=0).astype(np.float32)
    ls = np.concatenate(lss, axis=0).astype(np.float32)
    return y, ls
